# revision 15
# baseline (speedup 1.0000x reference)
"""Trainium2 Bass kernel for the entity-assignment loss.

Math: per sample b, C[i,j] = mean_d (yt[b,i,d]-yp[b,j,d])^2.
loss = mean_b ( min_perm sum_i C[i, perm(i)] / 8 ).

Since each permutation uses every row i and every column j exactly once,
  sum_i C[i, perm(i)] = (nt + np - 2 * sum_i dot(i, perm(i))) / 64
with nt+np a per-sample constant, so min over perms only needs MAX over
perms of the dot sum, computed with a 2^8 bitmask DP whose bit-i update
is a perfectly strided access pattern.

Engine split: DVE runs the fp16 2x dot-product chain and the DP max
updates; Pool (GpSimd) computes the DP candidate adds (tensor_scalar
with per-partition G column) for most update steps plus the k=0 copies;
Scalar does one fused cast and one fused squared-norm accum per chunk.

Sharding: pure data parallelism, 256 samples per core across 8 cores.
"""

import sys

if "/opt/trn_rl_repo" not in sys.path:
    sys.path.insert(0, "/opt/trn_rl_repo")

import numpy as np

B, N, D = 2048, 8, 64
N_CORES = 8
B_LOC = B // N_CORES        # 256 samples per core
NT = B_LOC // 128           # 2 partition tiles of 128 samples
NEG16 = -60000.0

TRACE = False
_CACHE = {}

# i-values whose candidate add runs on Scalar (activation with per-partition
# G bias) with the max on DVE as a fp16 2x tensor_tensor; the rest stay fused
# DVE STTs. i=0 has 1-element inner runs (no 2x) so it stays in the STT group.
SC_I = (3, 4, 5, 6, 7)


def _build():
    import concourse.bacc as bacc
    import concourse.mybir as mybir
    from concourse.tile import TileContext

    f32 = mybir.dt.float32
    f16 = mybir.dt.float16
    Alu = mybir.AluOpType
    Act = mybir.ActivationFunctionType

    nc = bacc.Bacc("TRN2", target_bir_lowering=False, debug=False)
    io_d = [nc.declare_dram_parameter(f"io{c}", [128, 2 * N * D], f16,
                                      isOutput=False) for c in range(NT)]
    out_d = nc.declare_dram_parameter("out", [128, NT], f32, isOutput=True)

    with TileContext(nc) as tc:
        with (
            tc.tile_pool(name="io", bufs=1) as io_pool,
            tc.tile_pool(name="work", bufs=2) as work_pool,
            tc.tile_pool(name="res", bufs=1) as res_pool,
        ):
            ioh = [io_pool.tile([128, 2 * N * D], f16, tag=f"io{c}", name=f"io{c}")
                   for c in range(NT)]
            loss_t = res_pool.tile([128, NT], f32, tag="loss", name="loss")
            s_all = res_pool.tile([128, NT], f32, tag="s_all", name="s_all")
            G32 = res_pool.tile([128, NT * N * N], f32, tag="G32", name="G32")
            sq = work_pool.tile([128, 2 * N * D], f32, tag="sq", name="sq")

            # input DMAs on separate engine queues (parallel descriptor gen,
            # chunk 0 first)
            nc.sync.dma_start(out=ioh[0][:, :], in_=io_d[0][:, :])
            nc.scalar.dma_start(out=ioh[1][:, :], in_=io_d[1][:, :])
            # per-chunk broadcast multiplies into one joint prod tile,
            # then a single joint fold tree + segmented reduce (the DP needs
            # both chunks' G anyway, so per-chunk folds buy no overlap)
            prod = work_pool.tile([128, NT * N * N * D], f16, tag="prod",
                                  name="prod")
            for c in range(NT):
                yt_t = ioh[c][:, 0:N * D]
                yp_t = ioh[c][:, N * D:2 * N * D]
                yt_b = yt_t.rearrange("p (i d) -> p i d", d=D).unsqueeze(2) \
                    .broadcast_to([128, N, N, D])
                yp_b = yp_t.rearrange("p (j d) -> p j d", d=D).unsqueeze(1) \
                    .broadcast_to([128, N, N, D])
                nc.vector.tensor_tensor(
                    out=prod[:, c * N * N * D:(c + 1) * N * N * D]
                        .rearrange("p (i j d) -> p i j d", j=N, d=D),
                    in0=yt_b, in1=yp_b, op=Alu.mult)
                nc.scalar.activation(out=sq[:, :], in_=ioh[c][:, :],
                                     func=Act.Square,
                                     accum_out=s_all[:, c:c + 1])
            pv = prod.rearrange("p (q d) -> p q d", d=D)
            half = work_pool.tile([128, NT * N * N * D // 2], f16, tag="half",
                                  name="half")
            hv = half.rearrange("p (q d) -> p q d", d=D // 2)
            nc.vector.tensor_tensor(
                out=hv, in0=pv[:, :, 0:D // 2], in1=pv[:, :, D // 2:D],
                op=Alu.add)
            quart = work_pool.tile([128, NT * N * N * D // 4], f16, tag="quart",
                                   name="quart")
            qv = quart.rearrange("p (q d) -> p q d", d=D // 4)
            nc.vector.tensor_tensor(
                out=qv, in0=hv[:, :, 0:D // 4], in1=hv[:, :, D // 4:D // 2],
                op=Alu.add)
            eighth = work_pool.tile([128, NT * N * N * D // 8], f16,
                                    tag="eighth", name="eighth")
            ev = eighth.rearrange("p (q d) -> p q d", d=D // 8)
            nc.vector.tensor_tensor(
                out=ev, in0=qv[:, :, 0:D // 8], in1=qv[:, :, D // 8:D // 4],
                op=Alu.add)
            nc.vector.tensor_reduce(
                out=G32[:, :], in_=ev, axis=mybir.AxisListType.X, op=Alu.add)

            # DP over both chunks jointly: states laid out [chunk, state]
            g_v = G32.rearrange("p (h q) -> p h q", h=NT)
            dpa = res_pool.tile([128, NT * 256], f16, tag="dpa", name="dpa")
            dpb = res_pool.tile([128, NT * 256], f16, tag="dpb", name="dpb")
            nc.gpsimd.memset(dpa[:, :], NEG16)
            nc.gpsimd.memset(dpb[:, :], NEG16)
            cand = res_pool.tile([128, NT * 128], f16, tag="cand", name="cand")
            # per-(k,i) Scalar candidate buffers (one per SC_I slot, per
            # chunk, double-buffered over k)
            pcand = [work_pool.tile([128, NT * 128], f16, tag=f"pc{t}",
                                    name=f"pc{t}")
                     for t in range(2 * len(SC_I))]
            bufs = [dpa, dpb]
            for k in range(N):
                old = bufs[k % 2]
                new = bufs[(k + 1) % 2]
                for i in range(N):
                    ci = 2 ** i
                    col = i * N + k
                    if k == 0:
                        # singletons, pairwise-merged: targets {2^i, 2^(i+1)}
                        # are stride-2^i; G cols {i*8, (i+1)*8} are stride-8.
                        if i % 2 == 1:
                            continue
                        nv = new.rearrange("p (h s) -> p h s", h=NT)
                        tgt = nv[:, :, ci:2 * ci + 1:ci]
                        gsrc = g_v[:, :, i * N:(i + 2) * N:N]
                        nc.vector.tensor_copy(tgt, gsrc)
                        continue
                    elif k == N - 1:
                        # final column: collect the 8 candidates densely; the
                        # max and the loss combine happen after the loop.
                        if i % 2 == 1:
                            continue
                        ov = old.rearrange("p (h s) -> p h s", h=NT)
                        src = ov[:, :, 255 - 2 * ci:256 - ci:ci]
                        cv = cand.rearrange("p (h s) -> p h s", h=NT)[:, :, i:i + 2]
                        gsrc = g_v[:, :, (i + 1) * N + k::-N][:, :, 0:2]
                        nc.vector.tensor_tensor(out=cv, in0=src, in1=gsrc,
                                                op=Alu.add)
                        continue
                    vo = old.rearrange("p (h a b c) -> p h a b c",
                                       h=NT, b=2, c=ci)
                    vn = new.rearrange("p (h a b c) -> p h a b c",
                                       h=NT, b=2, c=ci)
                    for h in range(NT):
                        src = vo[:, h, :, 0, :]
                        tgt = vn[:, h, :, 1, :]
                        gcol = G32[:, h * N * N + col:h * N * N + col + 1]
                        if i in SC_I:
                            # Scalar computes the candidate add; DVE maxes it
                            # in (fp16 2x; the h0/h1 chains interleave so the
                            # write-ack latency stays hidden)
                            pc = pcand[(k % 2) * len(SC_I) + SC_I.index(i)]
                            nc.scalar.activation(
                                out=pc[:, h * 128:(h + 1) * 128],
                                in_=src, func=Act.Identity, bias=gcol)
                            nc.vector.tensor_tensor(
                                out=tgt, in0=tgt,
                                in1=pc[:, h * 128:(h + 1) * 128]
                                    .rearrange("p (a c) -> p a c", c=ci),
                                op=Alu.max)
                        else:
                            nc.vector.scalar_tensor_tensor(
                                out=tgt, in0=src, scalar=gcol,
                                in1=tgt, op0=Alu.add, op1=Alu.max)
            dmax = res_pool.tile([128, NT], f16, tag="dmax", name="dmax")
            nc.vector.tensor_reduce(
                out=dmax[:, :],
                in_=cand.rearrange("p (h s) -> p h s", h=NT)[:, :, 0:N],
                axis=mybir.AxisListType.X, op=Alu.max)
            nc.vector.scalar_tensor_tensor(
                out=loss_t[:, :],
                in0=dmax[:, :],
                scalar=-2.0,
                in1=s_all[:, :],
                op0=Alu.mult,
                op1=Alu.add,
            )
            nc.sync.dma_start(out=out_d[:, :], in_=loss_t[:, :],
                              single_packet=True)
    nc.compile()
    return nc


def kernel(y_true: np.ndarray, y_pred: np.ndarray) -> np.ndarray:
    from concourse.bass_utils import run_bass_kernel_spmd

    if "nc" not in _CACHE:
        _CACHE["nc"] = _build()
    nc = _CACHE["nc"]

    yt = np.asarray(y_true, dtype=np.float32).reshape(B, N * D)
    yp = np.asarray(y_pred, dtype=np.float32).reshape(B, N * D)
    full = np.concatenate([yt, yp], axis=1).astype(np.float16)

    in_maps = [
        {
            f"io{c}": np.ascontiguousarray(
                full[cc * B_LOC + c * 128: cc * B_LOC + (c + 1) * 128])
            for c in range(NT)
        }
        for cc in range(N_CORES)
    ]
    res = run_bass_kernel_spmd(nc, in_maps, list(range(N_CORES)), trace=TRACE)
    _CACHE["last_results"] = res
    vals = np.concatenate([np.asarray(r["out"], dtype=np.float64).reshape(-1)
                           for r in res.results])
    loss = vals.mean() / (D * N)
    return np.float32(loss)
